# revision 1
# baseline (speedup 1.0000x reference)
"""Coordinate-descent (alternating Gauss-Seidel) kernel for Trainium2.

B=4 factorizations x ~ u @ v^T, M=N=4096, R=32.
  u_new = GS-sweep(a1 = x@v,   b1 = v^T v, u)
  v_new = GS-sweep(a2 = x^T@u_new, b2 = u_new^T u_new, v)

8 cores; core c owns rows [c*512,(c+1)*512) of all batches (u rows, and the
same n-range of v delivered by one fused ReduceScatter of partial a2/b2).

Pipeline: per batch: phase1 (stream x: cast bf16 -> persistent SBUF,
PE-transpose, dense a1 MM burst) -> per-batch u GS sweep -> phase2 partials
+ RS-input DMAs (all overlap the next batch's phase1). One ReduceScatter,
v-transposes hoisted before it, batched v GS sweep after.
"""

import os
from contextlib import ExitStack

import numpy as np

import concourse.bass as bass
import concourse.tile as tile
from concourse import bacc, mybir
from concourse.bass import ds
from concourse.bass_utils import run_bass_kernel_spmd
from concourse.masks import make_identity

B, M, N, R = 4, 4096, 4096, 32
NCORES = 8
MS = M // NCORES          # 512 rows per core per batch
MC = MS // 128            # 4 m-chunks of 128
NG = N // 512             # 8 n-groups of 512
NCH = N // 128            # 32 n-chunks of 128
BMC = B * MC              # 16
CHUNK = B * (MS + R)      # 2176 rows per core in the fused ReduceScatter
EPS = 1e-8
FP32 = mybir.dt.float32
BF16 = mybir.dt.bfloat16
ALU = mybir.AluOpType

_CACHE = {}
LAST_RESULT = None


def _gs_sweep(nc, gsp, pmisc, ident_f, u_ap, a_ap, b_sbs, ball_ap, nb,
              unew, pre_uT=None):
    """Gauss-Seidel sweep over nb batches at once.

    u_ap/a_ap/unew: [128, nb*MC, R] fp32 APs; b_sbs: nb [R,R] grams (SBUF);
    ball_ap: [128, nb, R, R] fp32 replicated grams; pre_uT: optional
    precomputed list of [R, MC, 128] transposed-factor tiles.
    """
    nmc = nb * MC
    s = gsp.tile([128, BMC, R], FP32, tag="s", name="s")[:, :nmc, :]
    for bb in range(nb):
        if pre_uT is None:
            puT = pmisc.tile([R, MC, 128], FP32, tag="pm")
            for i in range(MC):
                nc.tensor.transpose(puT[:, i], u_ap[:, bb * MC + i, :],
                                    ident_f)
            uT = gsp.tile([R, MC, 128], FP32, tag="uT")
            nc.vector.tensor_copy(uT[:], puT[:])
        else:
            uT = pre_uT[bb]
        ps = pmisc.tile([128, MC, R], FP32, tag="pm")
        for i in range(MC):
            nc.tensor.matmul(ps[:, i], lhsT=uT[:, i], rhs=b_sbs[bb][:],
                             start=True, stop=True)
        nc.vector.tensor_copy(s[:, bb * MC:(bb + 1) * MC, :], ps[:])

    brr16 = gsp.tile([128, BMC, R], FP32, tag="brr16", name="brr16")[:, :nmc, :]
    for bb in range(nb):
        diag_bc = bass.AP(ball_ap.tensor, ball_ap.offset + bb * R * R,
                          [ball_ap.ap[0], [0, MC], [R + 1, R]])
        nc.vector.tensor_copy(brr16[:, bb * MC:(bb + 1) * MC, :], diag_bc)
    inv16 = gsp.tile([128, BMC, R], FP32, tag="inv16", name="inv16")[:, :nmc, :]
    nc.vector.tensor_scalar_add(inv16[:], brr16[:], EPS)
    nc.vector.reciprocal(inv16[:], inv16[:])

    app = gsp.tile([128, BMC, R], FP32, tag="app", name="app")[:, :nmc, :]
    nc.vector.scalar_tensor_tensor(out=app[:], in0=a_ap, scalar=EPS,
                                   in1=inv16[:], op0=ALU.add, op1=ALU.mult)

    t1 = gsp.tile([128, BMC], FP32, tag="t1", name="t1")[:, :nmc]
    delta = gsp.tile([128, BMC, 1], FP32, tag="delta")
    tmp = gsp.tile([128, BMC, R - 1], FP32, tag="tmp")
    dap = delta[:]
    tap = tmp[:]
    for r in range(R):
        nc.vector.tensor_tensor(out=t1[:], in0=u_ap[:, :, r],
                                in1=brr16[:, :, r], op=ALU.mult)
        nc.vector.tensor_tensor(out=t1[:], in0=t1[:], in1=s[:, :, r],
                                op=ALU.subtract)
        nc.vector.tensor_tensor(out=t1[:], in0=t1[:], in1=inv16[:, :, r],
                                op=ALU.mult)
        nc.vector.tensor_tensor(out=unew[:, :, r], in0=t1[:],
                                in1=app[:, :, r], op=ALU.add)
        if r < R - 1:
            tail = R - 1 - r
            nc.vector.tensor_tensor(out=delta[:, :nmc, 0],
                                    in0=unew[:, :, r], in1=u_ap[:, :, r],
                                    op=ALU.subtract)
            d_bc = bass.AP(dap.tensor, dap.offset,
                           [dap.ap[0], [MC, nb], [1, MC], [0, tail]])
            brow_bc = bass.AP(ball_ap.tensor,
                              ball_ap.offset + r * R + r + 1,
                              [ball_ap.ap[0], [R * R, nb], [0, MC],
                               [1, tail]])
            t_out = bass.AP(tap.tensor, tap.offset,
                            [tap.ap[0], [MC * (R - 1), nb], [R - 1, MC],
                             [1, tail]])
            nc.vector.tensor_tensor(out=t_out, in0=d_bc, in1=brow_bc,
                                    op=ALU.mult)
            nc.vector.tensor_tensor(out=s[:, :, r + 1:],
                                    in0=s[:, :, r + 1:],
                                    in1=tmp[:, :nmc, :tail], op=ALU.add)


def _build():
    nc = bacc.Bacc("TRN2", target_bir_lowering=False, debug=False,
                   num_devices=NCORES)

    x_my = nc.dram_tensor("x_my", [B, MS, N], FP32, kind="ExternalInput").ap()
    u_my = nc.dram_tensor("u_my", [B, MS, R], FP32, kind="ExternalInput").ap()
    v_full = nc.dram_tensor("v_full", [B, N, R], FP32,
                            kind="ExternalInput").ap()
    v_my = nc.dram_tensor("v_my", [B, MS, R], FP32, kind="ExternalInput").ap()
    u_out = nc.dram_tensor("u_out", [B, MS, R], FP32,
                           kind="ExternalOutput").ap()
    v_out = nc.dram_tensor("v_out", [B, MS, R], FP32,
                           kind="ExternalOutput").ap()

    rs_in = nc.dram_tensor("rs_in", [NCORES * CHUNK, R], FP32)
    rs_out = nc.dram_tensor("rs_out", [CHUNK, R], FP32)
    b1_scr = nc.dram_tensor("b1_scr", [B, R, R], FP32)

    with tile.TileContext(nc) as tc, ExitStack() as ctx:
        const = ctx.enter_context(tc.tile_pool(name="const", bufs=1))
        big = ctx.enter_context(tc.tile_pool(name="big", bufs=1))
        xl = ctx.enter_context(tc.tile_pool(name="xl", bufs=4))
        xt = ctx.enter_context(tc.tile_pool(name="xt", bufs=1))
        xnatp = ctx.enter_context(tc.tile_pool(name="xnatp", bufs=2))
        vpool = ctx.enter_context(tc.tile_pool(name="vp", bufs=1))
        gsp = ctx.enter_context(tc.tile_pool(name="gsp", bufs=1))
        sm = ctx.enter_context(tc.tile_pool(name="sm", bufs=2))
        ppt = ctx.enter_context(tc.tile_pool(name="ppt", bufs=3, space="PSUM"))
        pa1p = ctx.enter_context(tc.tile_pool(name="pa1", bufs=1,
                                              space="PSUM"))
        pa2p = ctx.enter_context(tc.tile_pool(name="pa2", bufs=2,
                                              space="PSUM"))
        pmisc = ctx.enter_context(tc.tile_pool(name="pmisc", bufs=2,
                                               space="PSUM"))

        ident_b = const.tile([128, 128], BF16)
        make_identity(nc, ident_b)
        ident_f = const.tile([128, 128], FP32)
        make_identity(nc, ident_f)

        unew_all = big.tile([128, BMC, R], FP32)
        ball = big.tile([128, B, R, R], FP32)
        u_all = big.tile([128, BMC, R], FP32)
        a_all = big.tile([128, BMC, R], FP32)
        un_b = big.tile([128, BMC, R], BF16)

        b1_sbs = []
        xnat_tiles = {}
        for b in range(B):
            x_nat = xnatp.tile([128, MC, N], BF16, tag="xnat", name="xnat")
            xnat_tiles[b] = x_nat
            # ---------- v load + b1 = v^T v ----------
            v32 = vpool.tile([128, NCH, R], FP32, tag="v32")
            nc.sync.dma_start(v32[:],
                              v_full[b].rearrange("(c p) r -> p c r", p=128))
            vb = vpool.tile([128, NCH, R], BF16, tag="vb")
            nc.vector.tensor_copy(vb[:], v32[:])

            pb1 = pmisc.tile([R, R], FP32, tag="pm")
            for c in range(NCH):
                nc.tensor.matmul(pb1[:], lhsT=vb[:, c], rhs=vb[:, c],
                                 start=(c == 0), stop=(c == NCH - 1))
            b1_sb = sm.tile([R, R], FP32, tag=f"b1_{b}")
            nc.vector.tensor_copy(b1_sb[:], pb1[:])
            b1_sbs.append(b1_sb)
            nc.sync.dma_start(b1_scr.ap()[b], b1_sb[:])
            src = b1_scr.ap()[b]
            nc.sync.dma_start(
                ball[:, b], bass.AP(src.tensor, src.offset,
                                    [[0, 128], [R, R], [1, R]]))

            # ---------- phase 1: stream x, transpose; then dense MM burst ---
            xT = xt.tile([128, NCH, MS], BF16, tag="xT")
            for j in range(NG):
                for i in range(MC):
                    xload = xl.tile([128, 512], FP32, tag="xload")
                    nc.sync.dma_start(
                        xload[:],
                        x_my[b, i * 128:(i + 1) * 128, j * 512:(j + 1) * 512])
                    nc.scalar.copy(
                        x_nat[:, i, j * 512:(j + 1) * 512], xload[:])
                    pt = ppt.tile([128, 4, 128], BF16, tag="pt")
                    for k in range(4):
                        nc.tensor.transpose(
                            pt[:, k],
                            x_nat[:, i,
                                  (j * 4 + k) * 128:(j * 4 + k + 1) * 128],
                            ident_b)
                    nc.scalar.copy(
                        xT[:, j * 4:(j + 1) * 4, i * 128:(i + 1) * 128],
                        pt[:])
            pa1 = pa1p.tile([R, MS], FP32, tag="pa1")
            for c in range(NCH):
                nc.tensor.matmul(pa1[:], lhsT=vb[:, c], rhs=xT[:, c],
                                 start=(c == 0), stop=(c == NCH - 1))
            a1T_sb = sm.tile([R, MS], FP32, tag="a1T")
            nc.vector.tensor_copy(a1T_sb[:], pa1[:])

            # a natural + u load
            nc.sync.dma_start(u_all[:, b * MC:(b + 1) * MC, :],
                              u_my[b].rearrange("(i p) r -> p i r", p=128))
            pA = pmisc.tile([128, MC, R], FP32, tag="pm")
            for i in range(MC):
                nc.tensor.transpose(pA[:, i],
                                    a1T_sb[:, i * 128:(i + 1) * 128],
                                    ident_f[:R, :R])
            nc.vector.tensor_copy(a_all[:, b * MC:(b + 1) * MC, :], pA[:])

            # ---------- per-batch u GS sweep (overlaps next phase1) -------
            sl = slice(b * MC, (b + 1) * MC)
            _gs_sweep(nc, gsp, pmisc, ident_f, u_all[:, sl, :],
                      a_all[:, sl, :], [b1_sb], ball[:, b:b + 1],
                      1, unew_all[:, sl, :])
            nc.sync.dma_start(u_out[b].rearrange("(i p) r -> p i r", p=128),
                              unew_all[:, sl, :])
            nc.vector.tensor_copy(un_b[:, sl, :], unew_all[:, sl, :])

            # ---------- phase 2 partials + RS-input DMAs ------------------
            for g in range(NG):
                pa2 = pa2p.tile([128, 4, R], FP32, tag="pa2")
                for k in range(4):
                    nblk = g * 4 + k
                    for i in range(MC):
                        nc.tensor.matmul(
                            pa2[:, k],
                            lhsT=xnat_tiles[b][:, i,
                                       nblk * 128:(nblk + 1) * 128],
                            rhs=un_b[:, b * MC + i], start=(i == 0),
                            stop=(i == MC - 1))
                a2st = sm.tile([128, 4, R], FP32, tag="a2st")
                nc.vector.tensor_copy(a2st[:], pa2[:])
                dst = rs_in.ap()
                nc.sync.dma_start(
                    bass.AP(dst.tensor,
                            dst.offset + (g * CHUNK + b * (MS + R)) * R,
                            [[R, 128], [128 * R, 4], [1, R]]),
                    a2st[:])

            pb2 = pmisc.tile([R, R], FP32, tag="pm")
            for i in range(MC):
                nc.tensor.matmul(pb2[:], lhsT=un_b[:, b * MC + i],
                                 rhs=un_b[:, b * MC + i], start=(i == 0),
                                 stop=(i == MC - 1))
            b2st = sm.tile([R, R], FP32, tag="b2st")
            nc.vector.tensor_copy(b2st[:], pb2[:])
            for c in range(NCORES):
                nc.sync.dma_start(
                    rs_in.ap()[ds(c * CHUNK + b * (MS + R) + MS, R), :],
                    b2st[:])

        # ---------- v loads + transposes (overlap RS) ---------------------
        v_all = big.tile([128, BMC, R], FP32)
        vT_tiles = []
        for b in range(B):
            nc.sync.dma_start(v_all[:, b * MC:(b + 1) * MC, :],
                              v_my[b].rearrange("(i p) r -> p i r", p=128))
            pvT = pmisc.tile([R, MC, 128], FP32, tag="pm")
            for i in range(MC):
                nc.tensor.transpose(pvT[:, i], v_all[:, b * MC + i, :],
                                    ident_f)
            vT = sm.tile([R, MC, 128], FP32, tag=f"vT_{b}")
            nc.vector.tensor_copy(vT[:], pvT[:])
            vT_tiles.append(vT)

        nc.gpsimd.collective_compute(
            "ReduceScatter", ALU.add, replica_groups=[list(range(NCORES))],
            ins=[rs_in.ap()], outs=[rs_out.ap()])

        # ---------- batched v GS ------------------------------------------
        a2_all = big.tile([128, BMC, R], FP32)
        b2_sbs = []
        for b in range(B):
            nc.sync.dma_start(
                a2_all[:, b * MC:(b + 1) * MC, :],
                rs_out.ap()[ds(b * (MS + R), MS), :].rearrange(
                    "(i p) r -> p i r", p=128))
            b2_sb = sm.tile([R, R], FP32, tag=f"b2_{b}")
            nc.sync.dma_start(b2_sb[:],
                              rs_out.ap()[ds(b * (MS + R) + MS, R), :])
            b2_sbs.append(b2_sb)
            src = rs_out.ap()
            nc.sync.dma_start(
                ball[:, b],
                bass.AP(src.tensor, src.offset + (b * (MS + R) + MS) * R,
                        [[0, 128], [R, R], [1, R]]))

        vnew = big.tile([128, BMC, R], FP32)
        _gs_sweep(nc, gsp, pmisc, ident_f, v_all[:], a2_all[:], b2_sbs,
                  ball[:], B, vnew[:], pre_uT=vT_tiles)
        for b in range(B):
            nc.sync.dma_start(v_out[b].rearrange("(i p) r -> p i r", p=128),
                              vnew[:, b * MC:(b + 1) * MC, :])

    nc.compile()
    return nc


def kernel(x, u, v):
    global LAST_RESULT
    if "nc" not in _CACHE:
        _CACHE["nc"] = _build()
    nc = _CACHE["nc"]

    x = np.ascontiguousarray(x, dtype=np.float32)
    u = np.ascontiguousarray(u, dtype=np.float32)
    v = np.ascontiguousarray(v, dtype=np.float32)

    in_maps = []
    for c in range(NCORES):
        sl = slice(c * MS, (c + 1) * MS)
        in_maps.append({
            "x_my": np.ascontiguousarray(x[:, sl, :]),
            "u_my": np.ascontiguousarray(u[:, sl, :]),
            "v_full": v,
            "v_my": np.ascontiguousarray(v[:, sl, :]),
        })

    res = run_bass_kernel_spmd(nc, in_maps, list(range(NCORES)),
                               trace=os.environ.get("KBENCH_TRACE") == "1")
    LAST_RESULT = res
    u_new = np.concatenate([res.results[c]["u_out"] for c in range(NCORES)],
                           axis=1)
    v_new = np.concatenate([res.results[c]["v_out"] for c in range(NCORES)],
                           axis=1)
    return (u_new, v_new)



# revision 17
# speedup vs baseline: 1.7743x; 1.7743x over previous
"""Coordinate-descent (alternating Gauss-Seidel) kernel for Trainium2.

B=4 factorizations x ~ u @ v^T, M=N=4096, R=32.

Reformulation: the R-step Gauss-Seidel sweep equals a triangular solve:
  u_new = (a1 + eps - u_old @ L1) @ inv(triu(b1) + eps*I),  L1 = tril(b1,-1)
u-side triangular factors are host-precomputed from v (b1 = v^T v).
v-side factors come from b2 = u_new^T u_new, inverted exactly on-chip via
the nilpotent-squaring identity (I+M)^-1 = (I-M)(I+M^2)(I+M^4)(I+M^8)(I+M^16).

Cross-sharding kills both the on-chip transposes and the big ReduceScatter:
core c computes a1/u_new for m-rows [c*512,(c+1)*512) (consuming the
host-pretransposed xT column-slice), AllGathers the tiny u_new (32KB bf16),
then computes a2/v_new for n-rows [c*512,(c+1)*512) with the full m
contraction done locally on the natural-layout column slice of x.
All x data is cast to bf16 and laid out partition-major on the host, so
every big DMA is 128 fully-contiguous descriptors.
"""

import os
from contextlib import ExitStack

import numpy as np
from ml_dtypes import bfloat16

import concourse.bass as bass
import concourse.tile as tile
from concourse import bacc, mybir
from concourse.bass import ds
from concourse.bass_utils import run_bass_kernel_spmd
from concourse.masks import make_identity

B, M, N, R = 4, 4096, 4096, 32
NCORES = 8
MS = M // NCORES          # 512 rows per core
MC = MS // 128            # 4 chunks of 128 within the slice
NQ = M // 128             # 32 chunks of 128 over the full dim
NGRP = 4                  # PE column groups (tile_position packing)
CHG = NQ // NGRP          # 8 contraction chunks per column group
EPS = 1e-8
FP32 = mybir.dt.float32
BF16 = mybir.dt.bfloat16
ALU = mybir.AluOpType

COLTILE = True            # 4x column-packed PE bursts
STAGE = int(os.environ.get("KSTAGE", "99"))  # debug bisect: truncate pipeline

_CACHE = {}
LAST_RESULT = None


def _grouped_gram_burst(nc, out_ps, lhs_chunks, rhs_chunks):
    """Accumulate sum_q lhs[q]^T @ rhs[q] into 4 column groups of out_ps."""
    if COLTILE:
        for k in range(CHG):
            for g in range(NGRP):
                q = g * CHG + k
                nc.tensor.matmul(
                    out_ps[32 * g:32 * (g + 1), :], lhsT=lhs_chunks(q),
                    rhs=rhs_chunks(q), start=(k == 0), stop=(k == CHG - 1),
                    tile_position=(0, 32 * g), skip_group_check=True)
    else:
        for q in range(NQ):
            nc.tensor.matmul(out_ps[0:32, :], lhsT=lhs_chunks(q),
                             rhs=rhs_chunks(q), start=(q == 0),
                             stop=(q == NQ - 1))


def _group_sum(nc, wk, out_sb, out_ap, ps, free, tag):
    """out = sum of the 4 [32, free] column-group partials in ps.

    DVE may read only one PSUM operand per instruction, so chain the adds
    with the accumulator in SBUF.
    """
    if COLTILE:
        acc = wk.tile([32, free], FP32, tag=tag + "a")
        nc.vector.tensor_copy(acc[:], ps[0:32, :])
        nc.vector.tensor_tensor(out=acc[:], in0=acc[:], in1=ps[32:64, :],
                                op=ALU.add)
        nc.vector.tensor_tensor(out=acc[:], in0=acc[:], in1=ps[64:96, :],
                                op=ALU.add)
        nc.vector.tensor_tensor(out=out_ap, in0=acc[:], in1=ps[96:128, :],
                                op=ALU.add)
    else:
        nc.vector.tensor_copy(out_ap, ps[0:32, :])


def _build():
    nc = bacc.Bacc("TRN2", target_bir_lowering=False, debug=False,
                   num_devices=NCORES)

    xT_my = nc.dram_tensor("xT_my", [128, B * NQ * MS], BF16,
                           kind="ExternalInput").ap()
    x_cs = nc.dram_tensor("x_cs", [128, B * NQ * MS], BF16,
                          kind="ExternalInput").ap()
    v_bf = nc.dram_tensor("v_bf", [128, B * NQ * R], BF16,
                          kind="ExternalInput").ap()
    u_my = nc.dram_tensor("u_my", [128, B * MC * R], FP32,
                          kind="ExternalInput").ap()
    v_my = nc.dram_tensor("v_my", [128, B * MC * R], FP32,
                          kind="ExternalInput").ap()
    L1_d = nc.dram_tensor("L1", [B, R, R], FP32, kind="ExternalInput").ap()
    U1inv_d = nc.dram_tensor("U1inv", [B, R, R], FP32,
                             kind="ExternalInput").ap()
    I32_d = nc.dram_tensor("I32", [R, R], FP32, kind="ExternalInput").ap()
    UM_d = nc.dram_tensor("UM", [R, R], FP32, kind="ExternalInput").ap()
    LM_d = nc.dram_tensor("LM", [R, R], FP32, kind="ExternalInput").ap()
    ONES_d = nc.dram_tensor("ONES", [R, 1], FP32, kind="ExternalInput").ap()
    u_out = nc.dram_tensor("u_out", [128, B * MC * R], FP32,
                           kind="ExternalOutput").ap()
    v_out = nc.dram_tensor("v_out", [128, B * MC * R], FP32,
                           kind="ExternalOutput").ap()

    ag_in = nc.dram_tensor("ag_in", [B, MS * R], BF16)
    ag_out = nc.dram_tensor("ag_out", [B, NCORES * MS * R], BF16,
                            addr_space="Shared")

    with tile.TileContext(nc) as tc, ExitStack() as ctx:
        const = ctx.enter_context(tc.tile_pool(name="const", bufs=1))
        xTp = ctx.enter_context(tc.tile_pool(name="xTp", bufs=2))
        xcp = ctx.enter_context(tc.tile_pool(name="xcp", bufs=2))
        big = ctx.enter_context(tc.tile_pool(name="big", bufs=1))
        ubp = ctx.enter_context(tc.tile_pool(name="ubp", bufs=2))
        wk = ctx.enter_context(tc.tile_pool(name="wk", bufs=2))
        sm = ctx.enter_context(tc.tile_pool(name="sm", bufs=2))
        pap = ctx.enter_context(tc.tile_pool(name="pap", bufs=2,
                                             space="PSUM"))
        ptp = ctx.enter_context(tc.tile_pool(name="ptp", bufs=2,
                                             space="PSUM"))
        psp = ctx.enter_context(tc.tile_pool(name="psp", bufs=2,
                                             space="PSUM"))
        pnp = ctx.enter_context(tc.tile_pool(name="pnp", bufs=1,
                                             space="PSUM"))
        pb2p = ctx.enter_context(tc.tile_pool(name="pb2p", bufs=1,
                                              space="PSUM"))

        ident_f = const.tile([128, 128], FP32)
        make_identity(nc, ident_f)
        I32_t = const.tile([R, R], FP32)
        nc.sync.dma_start(I32_t[:], I32_d)
        UM_t = const.tile([R, R], FP32)
        nc.sync.dma_start(UM_t[:], UM_d)
        LM_t = const.tile([R, R], FP32)
        nc.sync.dma_start(LM_t[:], LM_d)
        ONES_t = const.tile([R, 1], FP32)
        nc.sync.dma_start(ONES_t[:], ONES_d)
        L1_ts, U1_ts = [], []
        for b in range(B):
            t = const.tile([R, R], FP32, name=f"L1_{b}")
            nc.sync.dma_start(t[:], L1_d[b])
            L1_ts.append(t)
            t = const.tile([R, R], FP32, name=f"U1_{b}")
            nc.sync.dma_start(t[:], U1inv_d[b])
            U1_ts.append(t)

        vb_t = big.tile([128, B, NQ, R], BF16)
        nc.sync.dma_start(vb_t[:].rearrange("p b q r -> p (b q r)"), v_bf)
        u_my_t = big.tile([128, B, MC, R], FP32)
        nc.sync.dma_start(u_my_t[:].rearrange("p b i r -> p (b i r)"), u_my)
        v_my_t = big.tile([128, B, MC, R], FP32)
        nc.sync.dma_start(v_my_t[:].rearrange("p b i r -> p (b i r)"), v_my)

        for b in range(B):
            # ------------- streaming loads of this batch's x slices --------
            xT_t = xTp.tile([128, NQ, MS], BF16, tag="xT")
            nc.sync.dma_start(xT_t[:].rearrange("p q m -> p (q m)"),
                              xT_my[:, ds(b * NQ * MS, NQ * MS)])
            xc_t = xcp.tile([128, NQ, MS], BF16, tag="xc")
            nc.sync.dma_start(xc_t[:].rearrange("p q m -> p (q m)"),
                              x_cs[:, ds(b * NQ * MS, NQ * MS)])

            # ------------- a1T = (x @ v)^T for my m-slice ------------------
            pa1 = pap.tile([128, MS], FP32, tag="pa")
            _grouped_gram_burst(nc, pa1,
                                lambda q: vb_t[:, b, q, :],
                                lambda q: xT_t[:, q, :])
            a1T_sb = wk.tile([32, MS], FP32, tag="a1T")
            _group_sum(nc, wk, a1T_sb, a1T_sb[:], pa1, MS, "g1")

            if STAGE < 2:
                continue
            # ------------- u triangular solve ------------------------------
            # a1 natural [128, MC, R]
            pa1n = psp.tile([128, MC, R], FP32, tag="ps")
            for i in range(MC):
                nc.tensor.transpose(pa1n[:, i], a1T_sb[:, i * 128:(i + 1) * 128],
                                    ident_f[:R, :R])
            # u_old^T
            puT = ptp.tile([32, MC, 128], FP32, tag="pt")
            for i in range(MC):
                nc.tensor.transpose(puT[:, i], u_my_t[:, b, i, :], ident_f)
            uT_sb = sm.tile([32, MC, 128], FP32, tag="uT")
            nc.scalar.copy(uT_sb[:], puT[:])
            # u_old @ L1
            puL = psp.tile([128, MC, R], FP32, tag="ps")
            for i in range(MC):
                nc.tensor.matmul(puL[:, i], lhsT=uT_sb[:, i], rhs=L1_ts[b][:],
                                 start=True, stop=True)
            # RHS = a1 + eps - u_old@L1
            a1n_sb = sm.tile([128, MC, R], FP32, tag="a1n")
            nc.scalar.copy(a1n_sb[:], pa1n[:])
            RHS_sb = sm.tile([128, MC, R], FP32, tag="RHS")
            nc.vector.scalar_tensor_tensor(out=RHS_sb[:], in0=a1n_sb[:],
                                           scalar=EPS, in1=puL[:],
                                           op0=ALU.add, op1=ALU.subtract)
            # RHS^T
            pRT = ptp.tile([32, MC, 128], FP32, tag="pt")
            for i in range(MC):
                nc.tensor.transpose(pRT[:, i], RHS_sb[:, i, :], ident_f)
            RT_sb = sm.tile([32, MC, 128], FP32, tag="RT")
            nc.scalar.copy(RT_sb[:], pRT[:])
            # u_new = RHS @ U1inv
            pUN = psp.tile([128, MC, R], FP32, tag="ps")
            for i in range(MC):
                nc.tensor.matmul(pUN[:, i], lhsT=RT_sb[:, i], rhs=U1_ts[b][:],
                                 start=True, stop=True)
            u_new_sb = sm.tile([128, MC, R], FP32, tag="un")
            nc.vector.tensor_copy(u_new_sb[:], pUN[:])
            u_new_bf = sm.tile([128, MC, R], BF16, tag="unb")
            nc.vector.tensor_copy(u_new_bf[:], pUN[:])
            nc.sync.dma_start(u_out[:, ds(b * MC * R, MC * R)],
                              u_new_sb[:].rearrange("p i r -> p (i r)"))

            if STAGE < 3:
                continue
            # ------------- AllGather u_new (bf16) --------------------------
            dst = ag_in.ap()
            nc.sync.dma_start(
                bass.AP(dst.tensor, dst.offset + b * MS * R,
                        [[R, 128], [128 * R, MC], [1, R]]),
                u_new_bf[:])
            nc.gpsimd.collective_compute(
                "AllGather", ALU.bypass,
                replica_groups=[list(range(NCORES))],
                ins=[ag_in.ap()[b]], outs=[ag_out.ap()[b]])
            ub_t = ubp.tile([128, NQ, R], BF16, tag="ub")
            src = ag_out.ap()
            nc.sync.dma_start(
                ub_t[:],
                bass.AP(src.tensor, src.offset + b * NCORES * MS * R,
                        [[R, 128], [128 * R, NQ], [1, R]]))

            if STAGE < 4:
                continue
            # ------------- a2T = (x^T @ u_new)^T for my n-slice ------------
            pa2 = pap.tile([128, MS], FP32, tag="pa")
            _grouped_gram_burst(nc, pa2,
                                lambda q: ub_t[:, q, :],
                                lambda q: xc_t[:, q, :])
            a2T_sb = wk.tile([32, MS], FP32, tag="a2T")
            _group_sum(nc, wk, a2T_sb, a2T_sb[:], pa2, MS, "g2")

            # ------------- b2 = u_new^T u_new ------------------------------
            pb2 = pb2p.tile([128, R], FP32, tag="pb2")
            _grouped_gram_burst(nc, pb2,
                                lambda q: ub_t[:, q, :],
                                lambda q: ub_t[:, q, :])
            b2_sb = sm.tile([R, R], FP32, tag="b2")
            _group_sum(nc, wk, b2_sb, b2_sb[:], pb2, R, "g3")

            if STAGE < 41:
                continue
            # ------------- exact inv(triu(b2)+eps I) via squaring ----------
            junk = sm.tile([R, R], FP32, tag="junk")
            nc.vector.tensor_tensor(out=junk[:], in0=b2_sb[:], in1=I32_t[:],
                                    op=ALU.mult)
            pdc = pnp.tile([R, R], FP32, tag="pn")
            nc.tensor.matmul(pdc[:, 0:1], lhsT=junk[:], rhs=ONES_t[:],
                             start=True, stop=True)
            dcol = sm.tile([R, 1], FP32, tag="dcol")
            nc.vector.tensor_scalar_add(dcol[:], pdc[:, 0:1], EPS)
            rd = sm.tile([R, 1], FP32, tag="rd")
            nc.vector.reciprocal(rd[:], dcol[:])
            Mm = sm.tile([R, R], FP32, tag="Mm")
            nc.vector.tensor_tensor(out=Mm[:], in0=b2_sb[:], in1=UM_t[:],
                                    op=ALU.mult)
            rd_bc = bass.AP(rd[:].tensor, rd[:].offset,
                            [rd[:].ap[0], [0, R]])
            nc.vector.tensor_tensor(out=Mm[:], in0=Mm[:], in1=rd_bc,
                                    op=ALU.mult)
            L2_sb = sm.tile([R, R], FP32, tag="L2")
            nc.vector.tensor_tensor(out=L2_sb[:], in0=b2_sb[:], in1=LM_t[:],
                                    op=ALU.mult)

            if STAGE < 42:
                continue

            def tr32(src_ap, tagn):
                ps = pnp.tile([R, R], FP32, tag="pn")
                nc.tensor.transpose(ps[:], src_ap, ident_f[:R, :R])
                t = sm.tile([R, R], FP32, tag=tagn)
                nc.vector.tensor_copy(t[:], ps[:])
                return t

            def mm32(lhsT_t, rhs_ap, tagn):
                ps = pnp.tile([R, R], FP32, tag="pn")
                nc.tensor.matmul(ps[:], lhsT=lhsT_t[:], rhs=rhs_ap,
                                 start=True, stop=True)
                t = sm.tile([R, R], FP32, tag=tagn)
                nc.vector.tensor_copy(t[:], ps[:])
                return t

            def add_I(src_t, tagn, sub=False):
                t = sm.tile([R, R], FP32, tag=tagn)
                if sub:
                    nc.vector.tensor_tensor(out=t[:], in0=I32_t[:],
                                            in1=src_t[:], op=ALU.subtract)
                else:
                    nc.vector.tensor_tensor(out=t[:], in0=src_t[:],
                                            in1=I32_t[:], op=ALU.add)
                return t

            # powers of M and their transposes
            Mt = tr32(Mm[:], "Mt")
            if STAGE < 43:
                continue
            M2 = mm32(Mt, Mm[:], "M2")
            M2t = tr32(M2[:], "M2t")
            M4 = mm32(M2t, M2[:], "M4")
            M4t = tr32(M4[:], "M4t")
            M8 = mm32(M4t, M4[:], "M8")
            M8t = tr32(M8[:], "M8t")
            M16 = mm32(M8t, M8[:], "M16")
            M16t = tr32(M16[:], "M16t")
            if STAGE < 44:
                continue
            # U2inv = (I-M)(I+M2)(I+M4)(I+M8)(I+M16) Dinv, right-associated
            Dinv = sm.tile([R, R], FP32, tag="Dinv")
            rd_bc2 = bass.AP(rd[:].tensor, rd[:].offset,
                             [rd[:].ap[0], [0, R]])
            nc.vector.tensor_tensor(out=Dinv[:], in0=I32_t[:], in1=rd_bc2,
                                    op=ALU.mult)
            T1 = mm32(add_I(M16t, "G4t"), Dinv[:], "T1")
            T2 = mm32(add_I(M8t, "G3t"), T1[:], "T2")
            T3 = mm32(add_I(M4t, "G2t"), T2[:], "T3")
            T4 = mm32(add_I(M2t, "G1t"), T3[:], "T4")
            U2inv = mm32(add_I(Mt, "G0t", sub=True), T4[:], "U2inv")

            if STAGE < 6:
                continue
            # ------------- v triangular solve ------------------------------
            pa2n = psp.tile([128, MC, R], FP32, tag="ps")
            for i in range(MC):
                nc.tensor.transpose(pa2n[:, i], a2T_sb[:, i * 128:(i + 1) * 128],
                                    ident_f[:R, :R])
            pvT = ptp.tile([32, MC, 128], FP32, tag="pt")
            for i in range(MC):
                nc.tensor.transpose(pvT[:, i], v_my_t[:, b, i, :], ident_f)
            vT_sb = sm.tile([32, MC, 128], FP32, tag="vT")
            nc.scalar.copy(vT_sb[:], pvT[:])
            pvL = psp.tile([128, MC, R], FP32, tag="ps")
            for i in range(MC):
                nc.tensor.matmul(pvL[:, i], lhsT=vT_sb[:, i], rhs=L2_sb[:],
                                 start=True, stop=True)
            a2n_sb = sm.tile([128, MC, R], FP32, tag="a2n")
            nc.scalar.copy(a2n_sb[:], pa2n[:])
            RHS2_sb = sm.tile([128, MC, R], FP32, tag="RHS2")
            nc.vector.scalar_tensor_tensor(out=RHS2_sb[:], in0=a2n_sb[:],
                                           scalar=EPS, in1=pvL[:],
                                           op0=ALU.add, op1=ALU.subtract)
            pRT2 = ptp.tile([32, MC, 128], FP32, tag="pt")
            for i in range(MC):
                nc.tensor.transpose(pRT2[:, i], RHS2_sb[:, i, :], ident_f)
            RT2_sb = sm.tile([32, MC, 128], FP32, tag="RT2")
            nc.scalar.copy(RT2_sb[:], pRT2[:])
            pVN = psp.tile([128, MC, R], FP32, tag="ps")
            for i in range(MC):
                nc.tensor.matmul(pVN[:, i], lhsT=RT2_sb[:, i], rhs=U2inv[:],
                                 start=True, stop=True)
            v_new_sb = sm.tile([128, MC, R], FP32, tag="vn")
            nc.vector.tensor_copy(v_new_sb[:], pVN[:])
            nc.sync.dma_start(v_out[:, ds(b * MC * R, MC * R)],
                              v_new_sb[:].rearrange("p i r -> p (i r)"))

    nc.compile()
    return nc


def _prep_inputs(x, u, v):
    """Host-side layout/precompute. Returns per-core in_maps."""
    x = np.ascontiguousarray(x, dtype=np.float32)
    u = np.ascontiguousarray(u, dtype=np.float32)
    v = np.ascontiguousarray(v, dtype=np.float32)

    xb = x.astype(bfloat16)
    # xT slice per core: [c, p, b, q, m_l] with n = q*128+p, m = c*512+m_l
    xT_all = np.ascontiguousarray(
        xb.reshape(B, NCORES, MS, NQ, 128).transpose(1, 4, 0, 3, 2)
    ).reshape(NCORES, 128, B * NQ * MS)
    # natural column slice per core: [c, p, b, i, n_l], m = i*128+p
    xcs_all = np.ascontiguousarray(
        xb.reshape(B, NQ, 128, NCORES, MS).transpose(3, 2, 0, 1, 4)
    ).reshape(NCORES, 128, B * NQ * MS)

    v_bf = np.ascontiguousarray(
        v.astype(bfloat16).reshape(B, NQ, 128, R).transpose(2, 0, 1, 3)
    ).reshape(128, B * NQ * R)
    u_all = np.ascontiguousarray(
        u.reshape(B, NCORES, MC, 128, R).transpose(1, 3, 0, 2, 4)
    ).reshape(NCORES, 128, B * MC * R)
    v_all = np.ascontiguousarray(
        v.reshape(B, NCORES, MC, 128, R).transpose(1, 3, 0, 2, 4)
    ).reshape(NCORES, 128, B * MC * R)

    v64 = v.astype(np.float64)
    b1 = np.einsum('bnr,bns->brs', v64, v64)
    L1 = np.tril(b1, -1).astype(np.float32)
    U1inv = np.stack([
        np.linalg.inv(np.triu(b1[b]) + EPS * np.eye(R)) for b in range(B)
    ]).astype(np.float32)

    I32 = np.eye(R, dtype=np.float32)
    UM = np.triu(np.ones((R, R), dtype=np.float32), 1)
    LM = np.tril(np.ones((R, R), dtype=np.float32), -1)
    ONES = np.ones((R, 1), dtype=np.float32)

    in_maps = []
    for c in range(NCORES):
        in_maps.append({
            "xT_my": xT_all[c],
            "x_cs": xcs_all[c],
            "v_bf": v_bf,
            "u_my": u_all[c],
            "v_my": v_all[c],
            "L1": L1,
            "U1inv": U1inv,
            "I32": I32,
            "UM": UM,
            "LM": LM,
            "ONES": ONES,
        })
    return in_maps


def kernel(x, u, v):
    global LAST_RESULT
    if "nc" not in _CACHE:
        _CACHE["nc"] = _build()
    nc = _CACHE["nc"]

    in_maps = _prep_inputs(x, u, v)
    res = run_bass_kernel_spmd(nc, in_maps, list(range(NCORES)),
                               trace=os.environ.get("KBENCH_TRACE") == "1")
    LAST_RESULT = res

    def assemble(key):
        arr = np.stack([res.results[c][key] for c in range(NCORES)])
        return np.ascontiguousarray(
            arr.reshape(NCORES, 128, B, MC, R).transpose(2, 0, 3, 1, 4)
        ).reshape(B, M, R)

    return (assemble("u_out"), assemble("v_out"))
